# revision 18
# baseline (speedup 1.0000x reference)
"""Sparse GQA attention (causal + sliding window + global tokens) with LoRA
projections and RoPE, distributed over 8 TRN2 NeuronCores.

Sharding: batch (2) x kv-head-group (4). Core (b, g) computes q heads
4g..4g+3 and kv head g for batch b, producing a partial output-projection
sum; the host adds the 4 group partials per batch.

Kernel structure (per core):
  Phase A: QKV projections (LoRA folded host-side) + rotate-half RoPE.
    Compute order v,k,q0..q3 per 512-token chunk; first chunk's x and the
    weights are DMA'd in fine pieces so matmuls start ~5us in.
  Phase B: attention with TRANSPOSED scores (keys on partitions, the four
    GQA heads packed into one 512-wide free dim), so one N=512 matmul per
    128-key block serves all heads and no P-transpose is needed.  The
    softmax denominator is a DVE add-tree over the exp'd tiles followed by
    a ones-vector matmul; 1/den is broadcast across partitions with a K=1
    matmul and applied during PSUM evacuation.  Work is software-pipelined
    three deep: QK/exp(qi) | PV/norm(qi-1) | out-proj(qi-2).
"""

import os
import sys

import numpy as np

for _p in ("/root/.axon_site", "/root/.axon_site/_ro/trn_rl_repo",
           "/root/.axon_site/_ro/pypackages", "/opt/trn_rl_repo"):
    if os.path.isdir(_p) and _p not in sys.path:
        sys.path.append(_p)

import ml_dtypes
import concourse.bacc as bacc
import concourse.mybir as mybir
import concourse.tile as tile
from concourse.bass_utils import run_bass_kernel_spmd

B, S, D = 2, 2048, 2048
H, KVH, HD = 16, 4, 128
WINDOW, GLOBAL = 512, 64
THETA = 1000000.0
NCORES = 8
GH = H // KVH          # q heads per core
GF = GH * HD           # 512 projection features per core
TOK = 512              # token chunk for projections
NCH = S // TOK
NQB = S // 128         # 16 query blocks
NKB = S // 128
WBL = WINDOW // 128    # 4 window blocks before the diagonal
INV_SQRT = 1.0 / float(np.sqrt(HD))
NEG = -1e9

F32 = mybir.dt.float32
BF16 = mybir.dt.bfloat16

_PROGRAM = {}


def _blocks_for(qi):
    """[(key_block, mask_name_or_None), ...] plus has_glob flag.
    Masked blocks come FIRST so their exp->mask-multiply chains run early,
    keeping the end of the softmax-sum off the critical path."""
    if qi == 0:
        return [(0, "diag")], False
    if qi <= 3:
        return ([(qi, "diag")]
                + [(kb, None) for kb in range(qi)]), False
    if qi == 4:
        return ([(4, "diag"), (0, "edgeg")]
                + [(kb, None) for kb in range(1, 4)]), False
    return ([(qi, "diag"), (qi - WBL, "edge")]
            + [(kb, None) for kb in range(qi - WBL + 1, qi)]), True


def _emit(nc, t):
    """Emit the per-core Tile program. `t` maps input names to DRAM APs."""
    tc = t["tc"]
    from contextlib import ExitStack

    xTd = t["xt"].ap()      # [NCH,128,16,TOK] chunk-packed
    wqT = t["wqt"].ap()     # [128,16,GF]
    wkT = t["wkt"].ap()     # [128,16,HD]
    wvT = t["wvt"].ap()
    woT = t["wot"].ap()     # [128,GH,D]
    y = t["y"].ap()         # [S, D]

    with ExitStack() as stk:
        singles = stk.enter_context(tc.tile_pool(name="singles", bufs=1))
        persist = stk.enter_context(tc.tile_pool(name="persist", bufs=1))

        ident_sb = singles.tile([128, 128], BF16)
        nc.sync.dma_start(out=ident_sb, in_=t["ident"].ap())
        ones_sb = singles.tile([128, 128], BF16)
        nc.sync.dma_start(out=ones_sb, in_=t["ones"].ap())
        diag_sb = singles.tile([128, 512], BF16)
        nc.sync.dma_start(out=diag_sb, in_=t["diagT"].ap())
        edge_sb = singles.tile([128, 512], BF16)
        nc.sync.dma_start(out=edge_sb, in_=t["edgeT"].ap())
        edgeg_sb = singles.tile([128, 512], BF16)
        nc.sync.dma_start(out=edgeg_sb, in_=t["edgegT"].ap())
        masks = {"diag": diag_sb, "edge": edge_sb, "edgeg": edgeg_sb}

        # trigger the exp table-set load while the PE waits on input DMA
        dmy = singles.tile([1, 2], F32)
        nc.vector.memset(dmy[:, 0:1], 0.0)
        nc.scalar.activation(dmy[:, 1:2], dmy[:, 0:1],
                             mybir.ActivationFunctionType.Exp)

        qT_sb = persist.tile([128, GH, S], BF16)    # [hd, head, tok]
        kT_sb = persist.tile([128, S], BF16)        # [hd, tok]
        v_sb = persist.tile([128, NKB, HD], BF16)   # [tok%128, kblock, hd]

        # ---------------- Phase A: projections + RoPE ----------------
        with ExitStack() as stka:
            wpool = stka.enter_context(tc.tile_pool(name="wpool", bufs=1))
            xpool = stka.enter_context(tc.tile_pool(name="xpool", bufs=2))
            tmp = stka.enter_context(tc.tile_pool(name="tmpa", bufs=2))
            psa = stka.enter_context(tc.tile_pool(name="psa", bufs=2,
                                                  space="PSUM"))

            # chunk 0 in four a-slices so the first matmul starts early
            xt0p = []
            for pz in range(4):
                tl = xpool.tile([128, 4, TOK], BF16, tag=f"xt0p{pz}", bufs=1)
                nc.sync.dma_start(out=tl, in_=xTd[0][:, pz * 4:pz * 4 + 4, :])
                xt0p.append(tl)
            wv_sb = wpool.tile([128, 16, HD], BF16)
            nc.sync.dma_start(out=wv_sb, in_=wvT)
            wk_sb = wpool.tile([128, 16, HD], BF16)
            nc.sync.dma_start(out=wk_sb, in_=wkT)
            cos_sb = wpool.tile([128, S], BF16)
            nc.sync.dma_start(out=cos_sb, in_=t["cos2t"].ap())
            sin_sb = wpool.tile([128, S], BF16)
            nc.sync.dma_start(out=sin_sb, in_=t["sins2t"].ap())
            wq_h = []
            for h in range(GH):
                tl = wpool.tile([128, 16, HD], BF16, tag=f"wqh{h}")
                nc.sync.dma_start(out=tl, in_=wqT[:, :, h * HD:(h + 1) * HD])
                wq_h.append(tl)

            def xsl(c, xt, a):
                if c == 0:
                    return xt0p[a // 4][:, a % 4, :]
                return xt[:, a, :]

            for c in range(NCH):
                cs = slice(c * TOK, (c + 1) * TOK)
                if c == 0:
                    xt = None
                else:
                    xt = xpool.tile([128, 16, TOK], BF16, tag="xt")
                    nc.sync.dma_start(out=xt, in_=xTd[c])

                # v: compute vT, then transpose to natural [tok, hd] bf16
                pv = psa.tile([128, TOK], F32, tag="pq")
                for a in range(16):
                    nc.tensor.matmul(pv, wv_sb[:, a, :], xsl(c, xt, a),
                                     start=(a == 0), stop=(a == 15))
                vt = tmp.tile([128, TOK], BF16, tag="vt")
                nc.scalar.copy(vt, pv)
                for b2 in range(TOK // 128):
                    vtp = psa.tile([128, 128], BF16, tag="vtp")
                    nc.tensor.transpose(vtp, vt[:, b2 * 128:(b2 + 1) * 128],
                                        ident_sb)
                    nc.vector.tensor_copy(v_sb[:, c * 4 + b2, :], vtp)

                # k then q0..q3, each with rotate-half RoPE
                for h in range(GH + 1):
                    if h == 0:
                        wap = wk_sb
                        dst = kT_sb[:, cs]
                    else:
                        wap = wq_h[h - 1]
                        dst = qT_sb[:, h - 1, cs]
                    pq = psa.tile([128, TOK], F32, tag="pq")
                    for a in range(16):
                        nc.tensor.matmul(pq, wap[:, a, :], xsl(c, xt, a),
                                         start=(a == 0), stop=(a == 15))
                    # t1 = rotate_half(pq) * sin_signed   (cross-partition)
                    t1 = tmp.tile([128, TOK], F32, tag="t1")
                    nc.vector.tensor_mul(t1[0:64, :], pq[64:128, :],
                                         sin_sb[0:64, cs])
                    nc.vector.tensor_mul(t1[64:128, :], pq[0:64, :],
                                         sin_sb[64:128, cs])
                    t2 = tmp.tile([128, TOK], F32, tag="t2")
                    nc.vector.tensor_mul(t2, pq, cos_sb[:, cs])
                    nc.vector.tensor_add(dst, t2, t1)

        # woT load overlaps attention (pool entered after phase-A release)
        wopool = tc.tile_pool(name="wopool", bufs=1)
        wopool_cm = wopool.__enter__()
        wo_sb = wopool_cm.tile([128, GH, D], BF16)
        nc.sync.dma_start(out=wo_sb, in_=woT)

        # ---- Phase B: transposed-score attention, 3-deep pipeline ----
        with ExitStack() as stkb:
            ppool = stkb.enter_context(tc.tile_pool(name="ppool", bufs=20))
            spool = stkb.enter_context(tc.tile_pool(name="spool", bufs=2))
            cpool = stkb.enter_context(tc.tile_pool(name="cpool", bufs=2))
            psb = stkb.enter_context(tc.tile_pool(name="psb", bufs=1,
                                                  space="PSUM"))

            def qk_stage(qi):
                """QK^T + exp for all 4 heads; masks are 0/1 multiplies on
                GpSimd; the running sum alternates between DVE and GpSimd."""
                blocks, has_glob = _blocks_for(qi)
                qs = slice(qi * 128, (qi + 1) * 128)
                q4 = qT_sb[:, :, qs]            # [128, GH, 128] strided rhs
                st = {"blocks": [], "glob": None, "s": None, "qi": qi}
                s_tile = None
                for bi, (kb, mk) in enumerate(blocks):
                    ps = psb.tile([128, 512], F32, tag="ps", bufs=3)
                    ks = slice(kb * 128, (kb + 1) * 128)
                    nc.tensor.matmul(ps, kT_sb[:, ks], q4,
                                     start=True, stop=True)
                    p = ppool.tile([128, 512], BF16, tag="p")
                    nc.scalar.activation(p, ps,
                                         mybir.ActivationFunctionType.Exp,
                                         scale=INV_SQRT)
                    if mk is not None:
                        nc.gpsimd.tensor_mul(p, p, masks[mk])
                    st["blocks"].append((kb, p))
                    # fold into the running sum as soon as each exp lands,
                    # alternating engines; later adds (nearer the critical
                    # path) go to the faster DVE
                    eng = nc.gpsimd if bi == 1 else (nc.vector if bi % 2 == 0
                                                     else nc.gpsimd)
                    if bi == 1:
                        s_tile = spool.tile([128, 512], BF16, tag="s")
                        eng.tensor_add(s_tile, st["blocks"][0][1], p)
                    elif bi > 1:
                        eng.tensor_add(s_tile, s_tile, p)
                if has_glob:
                    psg = psb.tile([128, 512], F32, tag="ps", bufs=3)
                    nc.tensor.matmul(psg[0:64, :], kT_sb[:, 0:64], q4,
                                     start=True, stop=True)
                    pg = ppool.tile([128, 512], BF16, tag="p")
                    nc.scalar.activation(pg[0:64, :], psg[0:64, :],
                                         mybir.ActivationFunctionType.Exp,
                                         scale=INV_SQRT)
                    st["glob"] = pg
                    nc.vector.tensor_add(s_tile[0:64, :], s_tile[0:64, :],
                                         pg[0:64, :])
                st["s"] = s_tile if s_tile is not None else st["blocks"][0][1]
                return st

            def den_stage(st):
                """Softmax denominator for all 4 heads: ones^T @ s.
                den shares its PSUM bank with invb (disjoint lifetimes)."""
                den = psb.tile([1, 512], F32, tag="dinv", bufs=1)
                nc.tensor.matmul(den, ones_sb[:, 0:1], st["s"],
                                 start=True, stop=True)
                inv = spool.tile([1, 512], BF16, tag="inv")
                with nc.allow_low_precision(reason="1/den in bf16; 0.4% rel "
                                            "err is within tolerance"):
                    nc.vector.reciprocal(inv, den)
                st["inv"] = inv

            def bcast_stage(st):
                """Broadcast 1/den across partitions; evacuate to SBUF."""
                invb = psb.tile([128, 512], F32, tag="dinv", bufs=1)
                nc.tensor.matmul(invb, ones_sb[0:1, :], st["inv"],
                                 start=True, stop=True)
                invb_sb = spool.tile([128, 512], BF16, tag="invb_sb", bufs=3)
                nc.scalar.copy(invb_sb, invb)
                st["invb_sb"] = invb_sb

            def pv_stage(st):
                """PV accumulate and normalized evacuation."""
                po = psb.tile([128, 512], F32, tag="po", bufs=2)
                n_mm = len(st["blocks"]) + (1 if st["glob"] is not None else 0)
                for i, (kb, p) in enumerate(st["blocks"]):
                    nc.tensor.matmul(po, v_sb[:, kb, :], p,
                                     start=(i == 0), stop=(i == n_mm - 1))
                if st["glob"] is not None:
                    nc.tensor.matmul(po, v_sb[0:64, 0, :],
                                     st["glob"][0:64, :],
                                     start=False, stop=True)
                oT = spool.tile([128, GF], BF16, tag="oT", bufs=3)
                nc.vector.tensor_mul(oT, po, st["invb_sb"])
                st["oT"] = oT

            def oproj_stage(st):
                """Output projection for this 128-token block."""
                qi = st["qi"]
                oT = st["oT"]
                ysb = cpool.tile([128, D], BF16, tag="ysb")
                for cc in range(4):
                    ns = slice(cc * 512, (cc + 1) * 512)
                    py = psb.tile([128, 512], F32, tag="py", bufs=2)
                    for hh in range(GH):
                        nc.tensor.matmul(py, oT[:, hh * 128:(hh + 1) * 128],
                                         wo_sb[:, hh, ns],
                                         start=(hh == 0), stop=(hh == GH - 1))
                    nc.vector.tensor_copy(ysb[:, ns], py)
                nc.sync.dma_start(out=y[qi * 128:(qi + 1) * 128, :], in_=ysb)

            stages = [None, None, None]  # [qi-1, qi-2, qi-3] states
            for qi in range(NQB + 3):
                prev1, prev2, prev3 = stages
                cur = qk_stage(qi) if qi < NQB else None
                if prev1 is not None:
                    den_stage(prev1)
                if prev2 is not None:
                    pv_stage(prev2)
                if prev3 is not None:
                    oproj_stage(prev3)
                if prev1 is not None:
                    bcast_stage(prev1)
                stages = [cur, prev1, prev2]

        wopool.__exit__(None, None, None)


def _build_program():
    if "nc" in _PROGRAM:
        return _PROGRAM["nc"]
    nc = bacc.Bacc("TRN2", target_bir_lowering=False, debug=False,
                   num_devices=NCORES)
    t = _declare(nc)
    with tile.TileContext(nc) as tc:
        t["tc"] = tc
        _emit(nc, t)
    nc.compile()
    _PROGRAM["nc"] = nc
    return nc


def _declare(nc):
    t = {}
    t["xt"] = nc.dram_tensor("xt", [NCH, 128, 16, TOK], BF16,
                             kind="ExternalInput")
    t["wqt"] = nc.dram_tensor("wqt", [128, 16, GF], BF16,
                              kind="ExternalInput")
    t["wkt"] = nc.dram_tensor("wkt", [128, 16, HD], BF16,
                              kind="ExternalInput")
    t["wvt"] = nc.dram_tensor("wvt", [128, 16, HD], BF16,
                              kind="ExternalInput")
    t["wot"] = nc.dram_tensor("wot", [128, GH, D], BF16,
                              kind="ExternalInput")
    t["cos2t"] = nc.dram_tensor("cos2t", [128, S], BF16, kind="ExternalInput")
    t["sins2t"] = nc.dram_tensor("sins2t", [128, S], BF16,
                                 kind="ExternalInput")
    t["ident"] = nc.dram_tensor("ident", [128, 128], BF16,
                                kind="ExternalInput")
    t["ones"] = nc.dram_tensor("ones", [128, 128], BF16,
                               kind="ExternalInput")
    t["diagT"] = nc.dram_tensor("diagT", [128, 512], BF16,
                                kind="ExternalInput")
    t["edgeT"] = nc.dram_tensor("edgeT", [128, 512], BF16,
                                kind="ExternalInput")
    t["edgegT"] = nc.dram_tensor("edgegT", [128, 512], BF16,
                                 kind="ExternalInput")
    t["y"] = nc.dram_tensor("y", [S, D], BF16, kind="ExternalOutput")
    return t


def _host_inputs(x, wq_w, wq_a, wq_b, wk_w, wk_a, wk_b, wv_w, wv_a, wv_b,
                 wo_w, wo_a, wo_b):
    f32 = np.float32
    bf16 = ml_dtypes.bfloat16
    Wq = (wq_w.astype(f32) + wq_b.astype(f32) @ wq_a.astype(f32))
    Wk = (wk_w.astype(f32) + wk_b.astype(f32) @ wk_a.astype(f32))
    Wv = (wv_w.astype(f32) + wv_b.astype(f32) @ wv_a.astype(f32))
    Wo = (wo_w.astype(f32) + wo_b.astype(f32) @ wo_a.astype(f32))

    perm = np.concatenate([np.arange(0, HD, 2), np.arange(1, HD, 2)])
    Wq_p = Wq.reshape(H, HD, D)[:, perm, :].reshape(H * HD, D)
    Wk_p = Wk.reshape(KVH, HD, D)[:, perm, :].reshape(KVH * HD, D)

    j = np.arange(HD // 2, dtype=np.float64)
    inv_freq = 1.0 / THETA ** (2.0 * j / HD)
    tpos = np.arange(S, dtype=np.float64)
    freqs = np.outer(inv_freq, tpos)                      # [64, S]
    cosT = np.cos(freqs)
    sinT = np.sin(freqs)
    cos2t = np.concatenate([cosT, cosT], 0).astype(bf16)
    sins2t = np.concatenate([-sinT, sinT], 0).astype(bf16)

    a = np.arange(128)
    # multiplicative 0/1 masks in TRANSPOSED [key, query] layout,
    # replicated x4 for the heads (applied to exp'd scores)
    diagT = np.where(a[:, None] <= a[None, :], 1.0, 0.0)
    edgeT = np.where(a[:, None] > a[None, :], 1.0, 0.0)
    edgegT = np.where((a[:, None] > a[None, :]) | (a[:, None] < GLOBAL),
                      1.0, 0.0)
    diagT = np.tile(diagT, (1, GH)).astype(bf16)
    edgeT = np.tile(edgeT, (1, GH)).astype(bf16)
    edgegT = np.tile(edgegT, (1, GH)).astype(bf16)
    ident = np.eye(128, dtype=bf16)
    ones = np.ones((128, 128), dtype=bf16)

    common = dict(cos2t=cos2t, sins2t=sins2t, diagT=diagT, edgeT=edgeT,
                  edgegT=edgegT, ident=ident, ones=ones)

    def pack_w(wT, nf):
        # [D, nf] -> [128, 16, nf], partition-contiguous
        return np.ascontiguousarray(
            wT.reshape(16, 128, nf).transpose(1, 0, 2)).astype(bf16)

    NCH_ = S // TOK
    in_maps = []
    for b in range(B):
        xT = x[b].astype(f32).T.astype(bf16)            # [D, S]
        xh = np.ascontiguousarray(
            xT.reshape(16, 128, NCH_, TOK).transpose(2, 1, 0, 3))
        for g in range(KVH):
            woT = Wo[:, GF * g:GF * (g + 1)].T          # [GF, D]
            woh = np.ascontiguousarray(
                woT.reshape(GH, 128, D).transpose(1, 0, 2)).astype(bf16)
            in_maps.append(dict(
                xt=xh,
                wqt=pack_w(Wq_p[GF * g:GF * (g + 1), :].T, GF),
                wkt=pack_w(Wk_p[HD * g:HD * (g + 1), :].T, HD),
                wvt=pack_w(Wv[HD * g:HD * (g + 1), :].T, HD),
                wot=woh,
                **common,
            ))
    return in_maps


def kernel(**inputs):
    nc = _build_program()
    in_maps = _host_inputs(**inputs)
    res = None
    last_err = None
    for _attempt in range(3):
        try:
            res = run_bass_kernel_spmd(nc, in_maps,
                                       core_ids=list(range(NCORES)))
            break
        except Exception as e:  # transient first-exec device hiccups
            last_err = e
            import time as _time
            _time.sleep(2.0)
    if res is None:
        raise last_err
    out = np.zeros((B, S, D), dtype=np.float32)
    for b in range(B):
        for g in range(KVH):
            out[b] += res.results[b * KVH + g]["y"].astype(np.float32)
    return out


# revision 24
# speedup vs baseline: 1.4002x; 1.4002x over previous
"""Sparse GQA attention (causal + sliding window + global tokens) with LoRA
projections and RoPE, distributed over 8 TRN2 NeuronCores.

Sharding: batch (2) x kv-head-group (4). Core (b, g) computes q heads
4g..4g+3 and kv head g for batch b, producing a partial output-projection
sum; the host adds the 4 group partials per batch.

Kernel structure (per core):
  Phase A: QKV projections (LoRA folded host-side) + rotate-half RoPE.
    Compute order v,k,q0..q3 per 512-token chunk; first chunk's x and the
    weights are DMA'd in fine pieces so matmuls start ~5us in.
  Phase B: attention with TRANSPOSED scores (keys on partitions, the four
    GQA heads packed into one 512-wide free dim), so one N=512 matmul per
    128-key block serves all heads and no P-transpose is needed.  The
    softmax denominator is a DVE add-tree over the exp'd tiles followed by
    a ones-vector matmul; 1/den is broadcast across partitions with a K=1
    matmul and applied during PSUM evacuation.  Work is software-pipelined
    three deep: QK/exp(qi) | PV/norm(qi-1) | out-proj(qi-2).
"""

import os
import sys

import numpy as np

for _p in ("/root/.axon_site", "/root/.axon_site/_ro/trn_rl_repo",
           "/root/.axon_site/_ro/pypackages", "/opt/trn_rl_repo"):
    if os.path.isdir(_p) and _p not in sys.path:
        sys.path.append(_p)

import ml_dtypes
import concourse.bacc as bacc
import concourse.mybir as mybir
import concourse.tile as tile
from concourse.bass_utils import run_bass_kernel_spmd

B, S, D = 2, 2048, 2048
H, KVH, HD = 16, 4, 128
WINDOW, GLOBAL = 512, 64
THETA = 1000000.0
NCORES = 8
GH = H // KVH          # q heads per core
GF = GH * HD           # 512 projection features per core
TOK = 512              # token chunk for projections
NCH = S // TOK
NQB = S // 128         # 16 query blocks
NKB = S // 128
WBL = WINDOW // 128    # 4 window blocks before the diagonal
INV_SQRT = 1.0 / float(np.sqrt(HD))
NEG = -1e9

F32 = mybir.dt.float32
BF16 = mybir.dt.bfloat16

_PROGRAM = {}


def _blocks_for(qi):
    """[(key_block, mask_name_or_None), ...] plus has_glob flag.
    Masked blocks come FIRST so their exp->mask-multiply chains run early,
    keeping the end of the softmax-sum off the critical path."""
    if qi == 0:
        return [(0, "diag")], False
    if qi <= 3:
        return ([(qi, "diag")]
                + [(kb, None) for kb in range(qi)]), False
    if qi == 4:
        return ([(4, "diag"), (0, "edgeg")]
                + [(kb, None) for kb in range(1, 4)]), False
    return ([(qi, "diag"), (qi - WBL, "edge")]
            + [(kb, None) for kb in range(qi - WBL + 1, qi)]), True


def _emit(nc, t):
    """Emit the per-core Tile program. `t` maps input names to DRAM APs."""
    tc = t["tc"]
    from contextlib import ExitStack

    xTd = t["xt"].ap()      # [NCH,128,16,TOK] chunk-packed
    wqT = t["wqt"].ap()     # [128,16,GF]
    wkT = t["wkt"].ap()     # [128,16,HD]
    wvT = t["wvt"].ap()
    woT = t["wot"].ap()     # [128,GH,D]
    y = t["y"].ap()         # [S, D]

    with ExitStack() as stk:
        singles = stk.enter_context(tc.tile_pool(name="singles", bufs=1))
        persist = stk.enter_context(tc.tile_pool(name="persist", bufs=1))

        # trigger the exp table-set load while the PE waits on input DMA
        dmy = singles.tile([1, 2], F32)
        nc.vector.memset(dmy[:, 0:1], 0.0)
        nc.scalar.activation(dmy[:, 1:2], dmy[:, 0:1],
                             mybir.ActivationFunctionType.Exp)

        qT_sb = persist.tile([128, GH, S], BF16)    # [hd, head, tok]
        kT_sb = persist.tile([128, S], BF16)        # [hd, tok]
        v_sb = persist.tile([128, NKB, HD], BF16)   # [tok%128, kblock, hd]

        # ---------------- Phase A: projections + RoPE ----------------
        with ExitStack() as stka:
            wpool = stka.enter_context(tc.tile_pool(name="wpool", bufs=1))
            xpool = stka.enter_context(tc.tile_pool(name="xpool", bufs=2))
            tmp = stka.enter_context(tc.tile_pool(name="tmpa", bufs=2))
            psa = stka.enter_context(tc.tile_pool(name="psa", bufs=2,
                                                  space="PSUM"))

            # DMA issue order is the lead-in critical path (~1us per issue
            # on Sync): first-consumed tensors go strictly first.
            xt0p = []
            for pz in range(4):
                xt0p_t = xpool.tile([128, 4, TOK], BF16, tag=f"xt0p{pz}",
                                    bufs=1)
                xt0p.append(xt0p_t)
            wv_sb = wpool.tile([128, 16, HD], BF16)
            wk_sb = wpool.tile([128, 16, HD], BF16)
            wq_h = []
            for h in range(GH):
                wq_h_t = wpool.tile([128, 16, HD], BF16, tag=f"wqh{h}")
                wq_h.append(wq_h_t)
            cossin_sb = wpool.tile([128, 2 * S], BF16)
            consts_sb = singles.tile([128, 1792], BF16)

            nc.sync.dma_start(out=xt0p[0], in_=xTd[0][:, 0:4, :])
            nc.sync.dma_start(out=xt0p[1], in_=xTd[0][:, 4:8, :])
            nc.sync.dma_start(out=wv_sb, in_=wvT)
            nc.sync.dma_start(out=xt0p[2], in_=xTd[0][:, 8:12, :])
            nc.sync.dma_start(out=xt0p[3], in_=xTd[0][:, 12:16, :])
            nc.sync.dma_start(out=wk_sb, in_=wkT)
            nc.sync.dma_start(out=wq_h[0], in_=wqT[:, :, 0:HD])
            nc.sync.dma_start(out=wq_h[1], in_=wqT[:, :, HD:2 * HD])
            nc.sync.dma_start(out=cossin_sb, in_=t["cossin"].ap())
            nc.sync.dma_start(out=wq_h[2], in_=wqT[:, :, 2 * HD:3 * HD])
            nc.sync.dma_start(out=wq_h[3], in_=wqT[:, :, 3 * HD:4 * HD])
            nc.sync.dma_start(out=consts_sb, in_=t["consts"].ap())

            cos_sb = cossin_sb[:, 0:S]
            sin_sb = cossin_sb[:, S:2 * S]
            ident_sb = consts_sb[:, 0:128]
            ones_sb = consts_sb[:, 128:256]
            masks = {"diag": consts_sb[:, 256:768],
                     "edge": consts_sb[:, 768:1280],
                     "edgeg": consts_sb[:, 1280:1792]}

            def xsl(c, xt, a):
                if c == 0:
                    return xt0p[a // 4][:, a % 4, :]
                return xt[:, a, :]

            for c in range(NCH):
                cs = slice(c * TOK, (c + 1) * TOK)
                if c == 0:
                    xt = None
                else:
                    xt = xpool.tile([128, 16, TOK], BF16, tag="xt")
                    nc.sync.dma_start(out=xt, in_=xTd[c])

                # v: compute vT, then transpose to natural [tok, hd] bf16
                pv = psa.tile([128, TOK], F32, tag="pq")
                for a in range(16):
                    nc.tensor.matmul(pv, wv_sb[:, a, :], xsl(c, xt, a),
                                     start=(a == 0), stop=(a == 15))
                vt = tmp.tile([128, TOK], BF16, tag="vt")
                nc.scalar.copy(vt, pv)
                for b2 in range(TOK // 128):
                    vtp = psa.tile([128, 128], BF16, tag="vtp")
                    nc.tensor.transpose(vtp, vt[:, b2 * 128:(b2 + 1) * 128],
                                        ident_sb)
                    nc.vector.tensor_copy(v_sb[:, c * 4 + b2, :], vtp)

                # k then q0..q3, each with rotate-half RoPE
                for h in range(GH + 1):
                    if h == 0:
                        wap = wk_sb
                        dst = kT_sb[:, cs]
                    else:
                        wap = wq_h[h - 1]
                        dst = qT_sb[:, h - 1, cs]
                    pq = psa.tile([128, TOK], F32, tag="pq")
                    for a in range(16):
                        nc.tensor.matmul(pq, wap[:, a, :], xsl(c, xt, a),
                                         start=(a == 0), stop=(a == 15))
                    # t1 = rotate_half(pq) * sin_signed   (cross-partition)
                    t1 = tmp.tile([128, TOK], F32, tag="t1")
                    nc.vector.tensor_mul(t1[0:64, :], pq[64:128, :],
                                         sin_sb[0:64, cs])
                    nc.vector.tensor_mul(t1[64:128, :], pq[0:64, :],
                                         sin_sb[64:128, cs])
                    t2 = tmp.tile([128, TOK], F32, tag="t2")
                    nc.vector.tensor_mul(t2, pq, cos_sb[:, cs])
                    nc.vector.tensor_add(dst, t2, t1)

        # woT load overlaps attention (pool entered after phase-A release)
        wopool = tc.tile_pool(name="wopool", bufs=1)
        wopool_cm = wopool.__enter__()
        wo_sb = wopool_cm.tile([128, GH, D], BF16)
        nc.sync.dma_start(out=wo_sb, in_=woT)

        # ---- Phase B: transposed-score attention, 3-deep pipeline ----
        with ExitStack() as stkb:
            ppool = stkb.enter_context(tc.tile_pool(name="ppool", bufs=20))
            spool = stkb.enter_context(tc.tile_pool(name="spool", bufs=2))
            cpool = stkb.enter_context(tc.tile_pool(name="cpool", bufs=2))
            psb = stkb.enter_context(tc.tile_pool(name="psb", bufs=1,
                                                  space="PSUM"))

            def qk_stage(qi):
                """QK^T + exp for all 4 heads; masks are 0/1 multiplies on
                GpSimd; the running sum alternates between DVE and GpSimd."""
                blocks, has_glob = _blocks_for(qi)
                qs = slice(qi * 128, (qi + 1) * 128)
                q4 = qT_sb[:, :, qs]            # [128, GH, 128] strided rhs
                st = {"blocks": [], "glob": None, "s": None, "qi": qi}
                s_tile = None
                for bi, (kb, mk) in enumerate(blocks):
                    ps = psb.tile([128, 512], F32, tag="ps", bufs=3)
                    ks = slice(kb * 128, (kb + 1) * 128)
                    nc.tensor.matmul(ps, kT_sb[:, ks], q4,
                                     start=True, stop=True)
                    p = ppool.tile([128, 512], BF16, tag="p")
                    nc.scalar.activation(p, ps,
                                         mybir.ActivationFunctionType.Exp,
                                         scale=INV_SQRT)
                    if mk is not None:
                        nc.gpsimd.tensor_mul(p, p, masks[mk])
                    st["blocks"].append((kb, p))
                    # fold into the running sum as soon as each exp lands,
                    # alternating engines; later adds (nearer the critical
                    # path) go to the faster DVE
                    eng = nc.gpsimd if bi == 1 else (nc.vector if bi % 2 == 0
                                                     else nc.gpsimd)
                    if bi == 1:
                        s_tile = spool.tile([128, 512], BF16, tag="s")
                        eng.tensor_add(s_tile, st["blocks"][0][1], p)
                    elif bi > 1:
                        eng.tensor_add(s_tile, s_tile, p)
                if has_glob:
                    psg = psb.tile([128, 512], F32, tag="ps", bufs=3)
                    nc.tensor.matmul(psg[0:64, :], kT_sb[:, 0:64], q4,
                                     start=True, stop=True)
                    pg = ppool.tile([128, 512], BF16, tag="p")
                    nc.scalar.activation(pg[0:64, :], psg[0:64, :],
                                         mybir.ActivationFunctionType.Exp,
                                         scale=INV_SQRT)
                    st["glob"] = pg
                    nc.vector.tensor_add(s_tile[0:64, :], s_tile[0:64, :],
                                         pg[0:64, :])
                st["s"] = s_tile if s_tile is not None else st["blocks"][0][1]
                return st

            def den_stage(st):
                """Softmax denominator, broadcast across partitions in the
                same matmul (all-ones stationary), then fast approx 1/x."""
                den = psb.tile([128, 512], F32, tag="dinv", bufs=1)
                nc.tensor.matmul(den, ones_sb, st["s"],
                                 start=True, stop=True)
                invb_sb = spool.tile([128, 512], F32, tag="invb_sb", bufs=2)
                nc.vector.reciprocal_approx_fast(invb_sb, den)
                st["invb_sb"] = invb_sb

            def pv_stage(st):
                """PV accumulate and normalized evacuation."""
                po = psb.tile([128, 512], F32, tag="po", bufs=2)
                n_mm = len(st["blocks"]) + (1 if st["glob"] is not None else 0)
                for i, (kb, p) in enumerate(st["blocks"]):
                    nc.tensor.matmul(po, v_sb[:, kb, :], p,
                                     start=(i == 0), stop=(i == n_mm - 1))
                if st["glob"] is not None:
                    nc.tensor.matmul(po, v_sb[0:64, 0, :],
                                     st["glob"][0:64, :],
                                     start=False, stop=True)
                oT = spool.tile([128, GF], BF16, tag="oT", bufs=3)
                nc.vector.tensor_mul(oT, po, st["invb_sb"])
                st["oT"] = oT

            def oproj_stage(st):
                """Output projection for this 128-token block."""
                qi = st["qi"]
                oT = st["oT"]
                ysb = cpool.tile([128, D], BF16, tag="ysb")
                for cc in range(4):
                    ns = slice(cc * 512, (cc + 1) * 512)
                    py = psb.tile([128, 512], F32, tag="py", bufs=2)
                    for hh in range(GH):
                        nc.tensor.matmul(py, oT[:, hh * 128:(hh + 1) * 128],
                                         wo_sb[:, hh, ns],
                                         start=(hh == 0), stop=(hh == GH - 1))
                    nc.vector.tensor_copy(ysb[:, ns], py)
                nc.sync.dma_start(out=y[qi * 128:(qi + 1) * 128, :], in_=ysb)

            stages = [None, None]  # [qi-1, qi-2] states
            for qi in range(NQB + 2):
                prev1, prev2 = stages
                cur = qk_stage(qi) if qi < NQB else None
                if prev1 is not None:
                    den_stage(prev1)
                    pv_stage(prev1)
                if prev2 is not None:
                    oproj_stage(prev2)
                stages = [cur, prev1]

        wopool.__exit__(None, None, None)


def _build_program():
    if "nc" in _PROGRAM:
        return _PROGRAM["nc"]
    nc = bacc.Bacc("TRN2", target_bir_lowering=False, debug=False,
                   num_devices=NCORES)
    t = _declare(nc)
    with tile.TileContext(nc) as tc:
        t["tc"] = tc
        _emit(nc, t)
    nc.compile()
    _PROGRAM["nc"] = nc
    return nc


def _declare(nc):
    t = {}
    t["xt"] = nc.dram_tensor("xt", [NCH, 128, 16, TOK], BF16,
                             kind="ExternalInput")
    t["wqt"] = nc.dram_tensor("wqt", [128, 16, GF], BF16,
                              kind="ExternalInput")
    t["wkt"] = nc.dram_tensor("wkt", [128, 16, HD], BF16,
                              kind="ExternalInput")
    t["wvt"] = nc.dram_tensor("wvt", [128, 16, HD], BF16,
                              kind="ExternalInput")
    t["wot"] = nc.dram_tensor("wot", [128, GH, D], BF16,
                              kind="ExternalInput")
    t["cossin"] = nc.dram_tensor("cossin", [128, 2 * S], BF16,
                                 kind="ExternalInput")
    t["consts"] = nc.dram_tensor("consts", [128, 1792], BF16,
                                 kind="ExternalInput")
    t["y"] = nc.dram_tensor("y", [S, D], BF16, kind="ExternalOutput")
    return t


def _host_inputs(x, wq_w, wq_a, wq_b, wk_w, wk_a, wk_b, wv_w, wv_a, wv_b,
                 wo_w, wo_a, wo_b):
    f32 = np.float32
    bf16 = ml_dtypes.bfloat16
    Wq = (wq_w.astype(f32) + wq_b.astype(f32) @ wq_a.astype(f32))
    Wk = (wk_w.astype(f32) + wk_b.astype(f32) @ wk_a.astype(f32))
    Wv = (wv_w.astype(f32) + wv_b.astype(f32) @ wv_a.astype(f32))
    Wo = (wo_w.astype(f32) + wo_b.astype(f32) @ wo_a.astype(f32))

    perm = np.concatenate([np.arange(0, HD, 2), np.arange(1, HD, 2)])
    Wq_p = Wq.reshape(H, HD, D)[:, perm, :].reshape(H * HD, D)
    Wk_p = Wk.reshape(KVH, HD, D)[:, perm, :].reshape(KVH * HD, D)

    j = np.arange(HD // 2, dtype=np.float64)
    inv_freq = 1.0 / THETA ** (2.0 * j / HD)
    tpos = np.arange(S, dtype=np.float64)
    freqs = np.outer(inv_freq, tpos)                      # [64, S]
    cosT = np.cos(freqs)
    sinT = np.sin(freqs)
    cos2t = np.concatenate([cosT, cosT], 0)
    sins2t = np.concatenate([-sinT, sinT], 0)
    cossin = np.concatenate([cos2t, sins2t], 1).astype(bf16)  # [128, 2S]

    a = np.arange(128)
    # multiplicative 0/1 masks in TRANSPOSED [key, query] layout,
    # replicated x4 for the heads (applied to exp'd scores)
    diagT = np.where(a[:, None] <= a[None, :], 1.0, 0.0)
    edgeT = np.where(a[:, None] > a[None, :], 1.0, 0.0)
    edgegT = np.where((a[:, None] > a[None, :]) | (a[:, None] < GLOBAL),
                      1.0, 0.0)
    consts = np.concatenate(
        [np.eye(128), np.ones((128, 128)), np.tile(diagT, (1, GH)),
         np.tile(edgeT, (1, GH)), np.tile(edgegT, (1, GH))], 1).astype(bf16)

    common = dict(cossin=cossin, consts=consts)

    def pack_w(wT, nf):
        # [D, nf] -> [128, 16, nf], partition-contiguous
        return np.ascontiguousarray(
            wT.reshape(16, 128, nf).transpose(1, 0, 2)).astype(bf16)

    NCH_ = S // TOK
    in_maps = []
    for b in range(B):
        xT = x[b].astype(f32).T.astype(bf16)            # [D, S]
        xh = np.ascontiguousarray(
            xT.reshape(16, 128, NCH_, TOK).transpose(2, 1, 0, 3))
        for g in range(KVH):
            woT = Wo[:, GF * g:GF * (g + 1)].T          # [GF, D]
            woh = np.ascontiguousarray(
                woT.reshape(GH, 128, D).transpose(1, 0, 2)).astype(bf16)
            in_maps.append(dict(
                xt=xh,
                wqt=pack_w(Wq_p[GF * g:GF * (g + 1), :].T, GF),
                wkt=pack_w(Wk_p[HD * g:HD * (g + 1), :].T, HD),
                wvt=pack_w(Wv[HD * g:HD * (g + 1), :].T, HD),
                wot=woh,
                **common,
            ))
    return in_maps


def kernel(**inputs):
    nc = _build_program()
    in_maps = _host_inputs(**inputs)
    res = None
    last_err = None
    for _attempt in range(3):
        try:
            res = run_bass_kernel_spmd(nc, in_maps,
                                       core_ids=list(range(NCORES)))
            break
        except Exception as e:  # transient first-exec device hiccups
            last_err = e
            import time as _time
            _time.sleep(2.0)
    if res is None:
        raise last_err
    out = np.zeros((B, S, D), dtype=np.float32)
    for b in range(B):
        for g in range(KVH):
            out[b] += res.results[b * KVH + g]["y"].astype(np.float32)
    return out
